# revision 23
# baseline (speedup 1.0000x reference)
"""MoE feed-forward (top-2 routing, E=8 experts) on 8 trn2 NeuronCores.

Strategy: token parallelism with zero collectives.
  - Core c owns tokens [1024*c, 1024*(c+1)) and computes their complete MoE
    output locally: router -> per-expert compaction -> all 8 experts' FFN on
    its own tokens -> scatter-add combine.  No cross-core communication.
  - Capacity drops never occur for this problem size (max expert load ~2151
    vs CAP 2560, 12 sigma of margin), so routing is purely local; a safety
    clamp drops any hypothetical overflow via DMA bounds checks.
  - Per (core, expert) token counts are ~256 +- 15; each expert group is
    padded to EPAD=384 slots (3 tiles of 128).  Padded slots compute token 0
    with gate 0 and contribute nothing.
  - FFN in bf16 (weights streamed from DRAM as pre-laid-out bf16 on the sync
    DMA queue, double-buffered; x gathered fp32 then cast during the
    transpose drain), fp32 PSUM accumulate.  Router logits in full fp32 to
    reproduce the reference's top-2 selection exactly.
  - Combine: gated expert-output rows are indirect-DMA scatter-ADDed
    (CCE add) straight into the zeroed fp32 output shard during the FFN,
    so there is no combine tail.

Token layout on-chip: [128 partitions, 8 columns], local token t = 128*c + p.
Slot r of expert e maps to slotmeta row (r'%128)*24 + r'//128, r' = e*384+r.
"""
import numpy as np

import concourse.tile as tile
from concourse import bass, bacc, mybir
from concourse.bass_utils import run_bass_kernel_spmd
from concourse.masks import make_identity, make_upper_triangular

N_CORES = 8
P = 128
E = 8
K = 2
D = 1024
F = 2048
B, S = 4, 2048
T = B * S                  # 8192 tokens
TPC = T // N_CORES         # 1024 tokens per core
NCOL = TPC // P            # 8 token columns
DC = D // P                # 8 d-chunks
FC = F // P                # 16 f-chunks
EPAD = 384                 # padded slots per expert (max observed count ~287)
ETIL = EPAD // P           # 3 slot tiles per expert
NSLOT = E * EPAD           # 3072 total slots
NSLOT_T = NSLOT // P       # 24 slot tiles
f32 = mybir.dt.float32
bf16 = mybir.dt.bfloat16
i32 = mybir.dt.int32
ADD = mybir.AluOpType.add
SUB = mybir.AluOpType.subtract
MUL = mybir.AluOpType.mult


def build_kernel():
    nc = bacc.Bacc(num_devices=N_CORES)

    # ---------------- parameters (host pre-laid-out) ----------------
    xb_s = nc.declare_dram_parameter("xb_s", [TPC, D], bf16, isOutput=False)
    xT_s = nc.declare_dram_parameter("xT_s", [P, DC * TPC], f32, isOutput=False)
    rw_t = nc.declare_dram_parameter("rw_t", [P, DC * E], f32, isOutput=False)
    rb_r = nc.declare_dram_parameter("rb_r", [P, NCOL * E], f32, isOutput=False)
    w1b = nc.declare_dram_parameter("w1b", [E, P, DC * FC * P], bf16, isOutput=False)
    w2b = nc.declare_dram_parameter("w2b", [E, P, FC * DC * P], bf16, isOutput=False)
    b1t = nc.declare_dram_parameter("b1t", [P, E * FC], f32, isOutput=False)
    b2t = nc.declare_dram_parameter("b2t", [P, E * DC], f32, isOutput=False)
    out_shard = nc.declare_dram_parameter("out_shard", [TPC, D], f32, isOutput=True)

    # ---------------- internal DRAM ----------------
    # four scatter tables so consecutive queue ops never share a WAW dep
    slotmetas = [nc.dram_tensor(f"slotmeta_{i}", [NSLOT, 2], f32) for i in range(4)]

    with tile.TileContext(nc) as tc:
        with tc.tile_pool(name="const", bufs=1) as cpool:
            rw_sb = cpool.tile([P, DC, E], f32)
            nc.sync.dma_start(out=rw_sb[:], in_=rw_t.ap().rearrange(
                "p (c e) -> p c e", c=DC))
            rb_sb = cpool.tile([P, NCOL * E], f32)
            nc.sync.dma_start(out=rb_sb[:], in_=rb_r.ap())
            b1_sb = cpool.tile([P, E * FC], f32)
            nc.sync.dma_start(out=b1_sb[:], in_=b1t.ap())
            b2_sb = cpool.tile([P, E * DC], f32)
            nc.sync.dma_start(out=b2_sb[:], in_=b2t.ap())

            ident = cpool.tile([P, P], f32)
            make_identity(nc, ident[:])
            identb = cpool.tile([P, P], bf16)
            nc.vector.tensor_copy(identb[:], ident[:])
            tri = cpool.tile([P, P], f32)
            make_upper_triangular(nc, tri[:], val=1.0, diag=False)  # tri[p,i]=1 iff p<i
            ones_col = cpool.tile([P, 1], f32)
            nc.gpsimd.memset(ones_col[:], 1.0)
            ones_row1 = cpool.tile([1, P], f32)
            nc.gpsimd.memset(ones_row1[:], 1.0)
            toki = cpool.tile([P, NCOL], i32)
            nc.gpsimd.iota(toki[:], pattern=[[P, NCOL]], base=0, channel_multiplier=1)
            tokf = cpool.tile([P, NCOL], f32)
            nc.vector.tensor_copy(tokf[:], toki[:])
            eidxi = cpool.tile([P, E * 2 * NCOL], i32)
            nc.gpsimd.iota(eidxi[:], pattern=[[1, E], [0, 2 * NCOL]], base=0,
                           channel_multiplier=0)
            eidxf = cpool.tile([P, E * 2 * NCOL], f32)
            nc.vector.tensor_copy(eidxf[:], eidxi[:])
            ebase = cpool.tile([P, E * 2 * NCOL], f32)
            nc.vector.tensor_scalar(out=ebase[:], in0=eidxf[:], scalar1=float(EPAD),
                                    scalar2=None, op0=MUL)
            # persistent routing results
            slot_tok = cpool.tile([P, NSLOT_T], i32)
            slot_w = cpool.tile([P, NSLOT_T], f32)

            with tc.tile_pool(name="w1p", bufs=2) as w1p, \
                 tc.tile_pool(name="w2p", bufs=2) as w2p:

                # =========== routing (local tokens only) ===========
                with tc.tile_pool(name="rt", bufs=1) as rt, \
                     tc.tile_pool(name="rtp", bufs=2, space="PSUM") as rtp:
                    # sync queue: xT first (router critical path), split per
                    # dci chunk so the logit matmuls pipeline with the load
                    xT_r = rt.tile([P, DC, TPC], f32)
                    for dci in range(DC):
                        nc.sync.dma_start(
                            out=xT_r[:, dci, :],
                            in_=xT_s.ap().rearrange("p (c t) -> p c t", c=DC)[:, dci, :])
                    # warm up the PE clock (HAM) with junk matmuls on the identity
                    wup = rtp.tile([P, P], f32, space="PSUM", tag="rps")
                    for w in range(24):
                        nc.tensor.matmul(out=wup[:], lhsT=ident[:], rhs=ident[:],
                                         start=(w == 0), stop=(w == 23))
                    # zero the output shard early (scalar queue, idle at start)
                    zot = rt.tile([P, D], f32)
                    nc.vector.memset(zot[:], 0.0)
                    for c in range(NCOL):
                        nc.scalar.dma_start(out=out_shard.ap()[c * P:(c + 1) * P, :],
                                            in_=zot[:])
                    lg_sb = rt.tile([E, TPC], f32)
                    for h in range(2):
                        lgp = rtp.tile([E, TPC // 2], f32, space="PSUM", tag="lgp")
                        for dci in range(DC):
                            nc.tensor.matmul(
                                out=lgp[:], lhsT=rw_sb[:, dci, :],
                                rhs=xT_r[:, dci, h * (TPC // 2):(h + 1) * (TPC // 2)],
                                start=(dci == 0), stop=(dci == DC - 1))
                        nc.vector.tensor_copy(lg_sb[:, h * (TPC // 2):(h + 1) * (TPC // 2)],
                                              lgp[:])
                    # batched top-2: transpose all columns, then wide DVE ops
                    lsb = rt.tile([P, NCOL * E], f32)
                    for c in range(NCOL):
                        ltp = rtp.tile([P, E], f32, space="PSUM", tag="ltp")
                        nc.tensor.transpose(out=ltp[:],
                                            in_=lg_sb[:, c * P:(c + 1) * P],
                                            identity=ident[0:E, 0:E])
                        nc.vector.tensor_copy(lsb[:, c * E:(c + 1) * E], ltp[:])
                    nc.vector.tensor_tensor(out=lsb[:], in0=lsb[:], in1=rb_sb[:],
                                            op=ADD)
                    mxa = rt.tile([P, NCOL * 8], f32)
                    mia = rt.tile([P, NCOL * 8], mybir.dt.uint32)
                    for c in range(NCOL):
                        nc.vector.max_with_indices(mxa[:, c * 8:(c + 1) * 8],
                                                   mia[:, c * 8:(c + 1) * 8],
                                                   lsb[:, c * E:(c + 1) * E])
                    E1f = rt.tile([P, NCOL], f32)
                    E2f = rt.tile([P, NCOL], f32)
                    nc.vector.tensor_copy(E1f[:], mia[:, 0:NCOL * 8:8])
                    nc.vector.tensor_copy(E2f[:], mia[:, 1:NCOL * 8:8])
                    G1 = rt.tile([P, NCOL], f32)
                    G2 = rt.tile([P, NCOL], f32)
                    diff = rt.tile([P, NCOL], f32)
                    nc.vector.tensor_tensor(out=diff[:], in0=mxa[:, 0:NCOL * 8:8],
                                            in1=mxa[:, 1:NCOL * 8:8], op=SUB)
                    nc.scalar.activation(out=G1[:], in_=diff[:],
                                         func=mybir.ActivationFunctionType.Sigmoid)
                    nc.vector.tensor_scalar(out=G2[:], in0=G1[:],
                                            scalar1=-1.0, scalar2=1.0,
                                            op0=MUL, op1=ADD)

                    # ----- batched per-expert ranks -----
                    NW = E * 2 * NCOL      # 128 work columns: e-major, j=(k,c)
                    erep = rt.tile([P, NW], f32)
                    for e in range(E):
                        nc.vector.tensor_copy(erep[:, e * 2 * NCOL:e * 2 * NCOL + NCOL],
                                              E1f[:])
                        nc.vector.tensor_copy(
                            erep[:, e * 2 * NCOL + NCOL:(e + 1) * 2 * NCOL], E2f[:])
                    mask = rt.tile([P, NW], f32)
                    nc.vector.tensor_tensor(out=mask[:], in0=erep[:], in1=eidxf[:],
                                            op=mybir.AluOpType.is_equal)
                    rps = rtp.tile([P, NW], f32, space="PSUM", tag="rps")
                    nc.tensor.matmul(out=rps[:], lhsT=tri[:], rhs=mask[:],
                                     start=True, stop=False)
                    cps = rtp.tile([1, NW], f32, space="PSUM", tag="cps")
                    nc.tensor.matmul(out=cps[:], lhsT=ones_col[:], rhs=mask[:],
                                     start=True, stop=True)
                    ctot = rt.tile([1, NW], f32)
                    nc.vector.tensor_copy(ctot[:], cps[:])
                    cinc = rt.tile([1, NW], f32)
                    for e in range(E):
                        sl = slice(e * 2 * NCOL, (e + 1) * 2 * NCOL)
                        nc.vector.tensor_tensor_scan(
                            out=cinc[:, sl], data0=ctot[:, sl], data1=ctot[:, sl],
                            initial=0.0, op0=ADD, op1=mybir.AluOpType.bypass)
                    cexc = rt.tile([1, NW], f32)
                    nc.vector.tensor_tensor(out=cexc[:], in0=cinc[:], in1=ctot[:],
                                            op=SUB)
                    nc.tensor.matmul(out=rps[:], lhsT=ones_row1[:], rhs=cexc[:],
                                     start=False, stop=True)
                    rank = rt.tile([P, NW], f32)
                    nc.vector.tensor_copy(rank[:], rps[:])
                    kept = rt.tile([P, NW], f32)
                    nc.vector.tensor_scalar(out=kept[:], in0=rank[:],
                                            scalar1=float(EPAD), scalar2=None,
                                            op0=mybir.AluOpType.is_lt)
                    valid = rt.tile([P, NW], f32)
                    nc.vector.tensor_tensor(out=valid[:], in0=mask[:], in1=kept[:],
                                            op=MUL)
                    sel = rt.tile([P, NW], f32)
                    nc.vector.tensor_tensor(out=sel[:], in0=rank[:], in1=ebase[:],
                                            op=ADD)
                    nc.vector.tensor_tensor(out=sel[:], in0=sel[:], in1=valid[:],
                                            op=MUL)
                    # acc[k] = slot index per assignment; vsum = 1 unless overflow
                    acc = rt.tile([P, 2 * NCOL], f32)
                    vsum = rt.tile([P, 2 * NCOL], f32)
                    nc.vector.memset(acc[:], 0.0)
                    nc.vector.memset(vsum[:], 0.0)
                    for e in range(E):
                        sl = slice(e * 2 * NCOL, (e + 1) * 2 * NCOL)
                        nc.vector.tensor_tensor(out=acc[:], in0=acc[:],
                                                in1=sel[:, sl], op=ADD)
                        nc.vector.tensor_tensor(out=vsum[:], in0=vsum[:],
                                                in1=valid[:, sl], op=ADD)
                    # overflow -> push dst far out of bounds (dropped by DMA)
                    nc.vector.tensor_scalar(out=vsum[:], in0=vsum[:],
                                            scalar1=-1.0e6, scalar2=1.0e6,
                                            op0=MUL, op1=ADD)
                    nc.vector.tensor_tensor(out=acc[:], in0=acc[:], in1=vsum[:],
                                            op=ADD)
                    # dst row = (slot%128)*24 + slot//128, per k chain so
                    # k=0 scatters start while k=1 DVE chain still runs
                    # padded slots: token 2048 (out of bounds -> DMA-dropped), gate 0
                    zslot = rt.tile([P, 2 * NSLOT_T], f32)
                    nc.gpsimd.memset(zslot[:, 0:2 * NSLOT_T:2], 2.0 * TPC)
                    nc.gpsimd.memset(zslot[:, 1:2 * NSLOT_T:2], 0.0)
                    for smt in slotmetas:
                        nc.gpsimd.dma_start(
                            out=smt.ap().rearrange("(p s) w -> p (s w)", p=P),
                            in_=zslot[:])
                    dsts, pays = [], []
                    for kk in range(2):
                        ak = slice(kk * NCOL, (kk + 1) * NCOL)
                        ri = rt.tile([P, NCOL], i32, tag="ri", bufs=2)
                        nc.vector.tensor_copy(ri[:], acc[:, ak])
                        rand_ = rt.tile([P, NCOL], i32, tag="rand", bufs=2)
                        nc.vector.tensor_scalar(out=rand_[:], in0=ri[:], scalar1=127,
                                                scalar2=None,
                                                op0=mybir.AluOpType.bitwise_and)
                        nc.vector.tensor_scalar(out=rand_[:], in0=rand_[:],
                                                scalar1=NSLOT_T, scalar2=None, op0=MUL)
                        rshr = rt.tile([P, NCOL], i32, tag="rshr", bufs=2)
                        nc.vector.tensor_scalar(out=rshr[:], in0=ri[:], scalar1=7,
                                                scalar2=None,
                                                op0=mybir.AluOpType.logical_shift_right)
                        dst = rt.tile([P, NCOL], i32, tag="dst", bufs=2)
                        nc.vector.tensor_tensor(out=dst[:], in0=rand_[:], in1=rshr[:],
                                                op=ADD)
                        pay = rt.tile([P, 2 * NCOL], f32, tag="pay", bufs=2)
                        nc.vector.tensor_copy(pay[:, 0:2 * NCOL:2], tokf[:])
                        nc.vector.tensor_copy(pay[:, 1:2 * NCOL:2],
                                              G1[:] if kk == 0 else G2[:])
                        dsts.append(dst)
                        pays.append(pay)
                    # rotate across four tables so consecutive queue entries
                    # have no write-after-write dependency and pipeline fully
                    opi = 0
                    for j in range(NCOL):
                        for kk in (0, 1):
                            nc.gpsimd.indirect_dma_start(
                                out=slotmetas[opi % 4].ap(),
                                out_offset=bass.IndirectOffsetOnAxis(
                                    ap=dsts[kk][:, j:j + 1], axis=0),
                                in_=pays[kk][:, 2 * j:2 * j + 2],
                                in_offset=None,
                                bounds_check=NSLOT - 1,
                                oob_is_err=False,
                            )
                            opi += 1
                    smfs = []
                    for i, smt in enumerate(slotmetas):
                        smf = rt.tile([P, NSLOT_T, 2], f32, tag=f"smf{i}")
                        nc.gpsimd.dma_start(out=smf[:], in_=smt.ap().rearrange(
                            "(p s) w -> p s w", p=P))
                        smfs.append(smf)
                    # each slot was written by exactly one table; the others
                    # hold the pad (tok=2048, gate=0): tok=min, gate=sum
                    tokm = rt.tile([P, NSLOT_T], f32)
                    gsum = rt.tile([P, NSLOT_T], f32)
                    nc.vector.tensor_tensor(out=tokm[:], in0=smfs[0][:, :, 0],
                                            in1=smfs[1][:, :, 0],
                                            op=mybir.AluOpType.min)
                    nc.vector.tensor_tensor(out=gsum[:], in0=smfs[0][:, :, 1],
                                            in1=smfs[1][:, :, 1], op=ADD)
                    for i in (2, 3):
                        nc.vector.tensor_tensor(out=tokm[:], in0=tokm[:],
                                                in1=smfs[i][:, :, 0],
                                                op=mybir.AluOpType.min)
                        nc.vector.tensor_tensor(out=gsum[:], in0=gsum[:],
                                                in1=smfs[i][:, :, 1], op=ADD)
                    nc.vector.tensor_copy(slot_tok[:], tokm[:])
                    nc.vector.tensor_copy(slot_w[:], gsum[:])

                # =========== expert FFN + scatter-add combine ===========
                with tc.tile_pool(name="ffn", bufs=1) as ffn, \
                     tc.tile_pool(name="xgp", bufs=2) as xgp, \
                     tc.tile_pool(name="xtp", bufs=2) as xtp, \
                     tc.tile_pool(name="ffp", bufs=2, space="PSUM") as ffp:

                    def issue_gather(e):
                        xg3 = xgp.tile([P, ETIL, D], bf16, tag="xg3")
                        for t in range(ETIL):
                            nc.gpsimd.indirect_dma_start(
                                out=xg3[:, t, :], out_offset=None,
                                in_=xb_s.ap(),
                                in_offset=bass.IndirectOffsetOnAxis(
                                    ap=slot_tok[:, e * ETIL + t:e * ETIL + t + 1],
                                    axis=0),
                                bounds_check=TPC - 1,
                                oob_is_err=False,
                            )
                        return xg3

                    def xgT_build(xg3):
                        # 8 bf16 transposes into one 1-bank psum tile, then a
                        # single wide DVE drain per slot tile
                        xgT = xtp.tile([P, DC, EPAD], bf16, tag="xgT")
                        for t in range(ETIL):
                            tpb = ffp.tile([P, DC, P], bf16, space="PSUM", tag="tpb")
                            for dci in range(DC):
                                nc.tensor.transpose(
                                    out=tpb[:, dci, :],
                                    in_=xg3[:, t, dci * P:(dci + 1) * P],
                                    identity=identb[:])
                            nc.vector.tensor_copy(xgT[:, :, t * P:(t + 1) * P],
                                                  tpb[:])
                        return xgT

                    xg3_cur = issue_gather(0)
                    xgT_cur = xgT_build(xg3_cur)
                    for e in range(E):
                        w1t = w1p.tile([P, DC, FC, P], bf16, tag="w1")
                        nc.sync.dma_start(out=w1t[:], in_=w1b.ap()[e].rearrange(
                            "p (a b q) -> p a b q", a=DC, b=FC))
                        w2t = w2p.tile([P, FC, DC, P], bf16, tag="w2")
                        nc.sync.dma_start(out=w2t[:], in_=w2b.ap()[e].rearrange(
                            "p (a b q) -> p a b q", a=FC, b=DC))
                        if e + 1 < E:
                            xg3_nxt = issue_gather(e + 1)

                        # mm1 + gelu -> hT
                        hT = ffn.tile([P, FC, EPAD], bf16, tag="hT", bufs=1)
                        for fci in range(FC):
                            hp = ffp.tile([P, EPAD], f32, space="PSUM", tag="hp")
                            for dci in range(DC):
                                nc.tensor.matmul(out=hp[:],
                                                 lhsT=w1t[:, dci, fci, :],
                                                 rhs=xgT_cur[:, dci, :],
                                                 start=(dci == 0), stop=(dci == DC - 1))
                            nc.scalar.activation(out=hT[:, fci, :], in_=hp[:],
                                                 func=mybir.ActivationFunctionType.Gelu,
                                                 bias=b1_sb[:, e * FC + fci:e * FC + fci + 1],
                                                 scale=1.0)
                        # next expert's input transposes; their DVE drains
                        # overlap this expert's mm2
                        if e + 1 < E:
                            xgT_nxt = xgT_build(xg3_nxt)

                        # mm2 (+bias) -> oT
                        oT = ffn.tile([P, DC, EPAD], bf16, tag="oT", bufs=1)
                        for dci in range(DC):
                            op = ffp.tile([P, EPAD], f32, space="PSUM", tag="op")
                            for fci in range(FC):
                                nc.tensor.matmul(out=op[:],
                                                 lhsT=w2t[:, fci, dci, :],
                                                 rhs=hT[:, fci, :],
                                                 start=(fci == 0), stop=(fci == FC - 1))
                            nc.vector.tensor_scalar(
                                out=oT[:, dci, :], in0=op[:],
                                scalar1=b2_sb[:, e * DC + dci:e * DC + dci + 1],
                                scalar2=None, op0=ADD)
                        # transpose back + gate (one DVE drain per slot tile),
                        # scatter-add into the output shard
                        ow3 = ffn.tile([P, ETIL, D], f32, tag="ow3", bufs=2)
                        for cc in range(ETIL):
                            tp2b = ffp.tile([P, DC, P], bf16, space="PSUM", tag="tp2b")
                            for dci in range(DC):
                                nc.tensor.transpose(
                                    out=tp2b[:, dci, :],
                                    in_=oT[:, dci, cc * P:(cc + 1) * P],
                                    identity=identb[:])
                            nc.vector.tensor_scalar(
                                out=ow3[:, cc, :], in0=tp2b[:],
                                scalar1=slot_w[:, e * ETIL + cc:e * ETIL + cc + 1],
                                scalar2=None, op0=MUL)
                            nc.gpsimd.indirect_dma_start(
                                out=out_shard.ap(),
                                out_offset=bass.IndirectOffsetOnAxis(
                                    ap=slot_tok[:, e * ETIL + cc:e * ETIL + cc + 1],
                                    axis=0),
                                in_=ow3[:, cc, :],
                                in_offset=None,
                                bounds_check=TPC - 1,
                                oob_is_err=False,
                                compute_op=ADD,
                            )
                        if e + 1 < E:
                            xg3_cur, xgT_cur = xg3_nxt, xgT_nxt

    nc.finalize()
    return nc


_NC_CACHE = None
TRACE = False
LAST_EXEC_NS = None
LAST_TRACE_DIR = None


def kernel(x, router_w, router_b, w1, b1, w2, b2):
    global _NC_CACHE, LAST_EXEC_NS, LAST_TRACE_DIR
    import ml_dtypes
    bf = ml_dtypes.bfloat16

    x = np.ascontiguousarray(np.asarray(x, np.float32))
    router_w = np.ascontiguousarray(np.asarray(router_w, np.float32))
    router_b = np.asarray(router_b, np.float32)
    w1 = np.asarray(w1, np.float32)
    b1 = np.asarray(b1, np.float32)
    w2 = np.asarray(w2, np.float32)
    b2 = np.asarray(b2, np.float32)

    xf = x.reshape(T, D)
    rb_r = np.ascontiguousarray(np.tile(router_b, (P, NCOL)))
    rw_t = np.ascontiguousarray(
        router_w.reshape(DC, P, E).transpose(1, 0, 2).reshape(P, DC * E))
    w1b = np.ascontiguousarray(
        w1.reshape(E, DC, P, FC, P).transpose(0, 2, 1, 3, 4)
        .reshape(E, P, DC * FC * P).astype(bf))
    w2b = np.ascontiguousarray(
        w2.reshape(E, FC, P, DC, P).transpose(0, 2, 1, 3, 4)
        .reshape(E, P, FC * DC * P).astype(bf))
    b1t = np.ascontiguousarray(
        b1.reshape(E, FC, P).transpose(2, 0, 1).reshape(P, E * FC))
    b2t = np.ascontiguousarray(
        b2.reshape(E, DC, P).transpose(2, 0, 1).reshape(P, E * DC))

    in_maps = []
    for c in range(N_CORES):
        xs = np.ascontiguousarray(xf[c * TPC:(c + 1) * TPC])
        xT = np.ascontiguousarray(
            xs.T.reshape(DC, P, TPC).transpose(1, 0, 2).reshape(P, DC * TPC))
        in_maps.append({
            "xb_s": xs.astype(bf),
            "xT_s": xT,
            "rw_t": rw_t,
            "rb_r": rb_r,
            "w1b": w1b,
            "w2b": w2b,
            "b1t": b1t,
            "b2t": b2t,
        })

    if _NC_CACHE is None:
        _NC_CACHE = build_kernel()
    import tempfile
    td = tempfile.mkdtemp(prefix="moe_trace_") if TRACE else None
    res = run_bass_kernel_spmd(_NC_CACHE, in_maps, list(range(N_CORES)),
                               trace=TRACE, tmpdir=td)
    LAST_EXEC_NS = getattr(res, "exec_time_ns", None)
    LAST_TRACE_DIR = td
    out = np.concatenate([res.results[c]["out_shard"] for c in range(N_CORES)], axis=0)
    return out.reshape(B, S, D)


# revision 24
# speedup vs baseline: 1.0115x; 1.0115x over previous
"""MoE feed-forward (top-2 routing, E=8 experts) on 8 trn2 NeuronCores.

Strategy: token parallelism with zero collectives.
  - Core c owns tokens [1024*c, 1024*(c+1)) and computes their complete MoE
    output locally: router -> per-expert compaction -> all 8 experts' FFN on
    its own tokens -> scatter-add combine.  No cross-core communication.
  - Capacity drops never occur for this problem size (max expert load ~2151
    vs CAP 2560, 12 sigma of margin), so routing is purely local; a safety
    clamp drops any hypothetical overflow via DMA bounds checks.
  - Per (core, expert) token counts are ~256 +- 15; each expert group is
    padded to EPAD=384 slots (3 tiles of 128).  Padded slots compute token 0
    with gate 0 and contribute nothing.
  - FFN in bf16 (weights streamed from DRAM as pre-laid-out bf16 on the sync
    DMA queue, double-buffered; x gathered fp32 then cast during the
    transpose drain), fp32 PSUM accumulate.  Router logits in full fp32 to
    reproduce the reference's top-2 selection exactly.
  - Combine: gated expert-output rows are indirect-DMA scatter-ADDed
    (CCE add) straight into the zeroed fp32 output shard during the FFN,
    so there is no combine tail.

Token layout on-chip: [128 partitions, 8 columns], local token t = 128*c + p.
Slot r of expert e maps to slotmeta row (r'%128)*24 + r'//128, r' = e*384+r.
"""
import numpy as np

import concourse.tile as tile
from concourse import bass, bacc, mybir
from concourse.bass_utils import run_bass_kernel_spmd
from concourse.masks import make_identity, make_upper_triangular

N_CORES = 8
P = 128
E = 8
K = 2
D = 1024
F = 2048
B, S = 4, 2048
T = B * S                  # 8192 tokens
TPC = T // N_CORES         # 1024 tokens per core
NCOL = TPC // P            # 8 token columns
DC = D // P                # 8 d-chunks
FC = F // P                # 16 f-chunks
EPAD = 384                 # padded slots per expert (max observed count ~287)
ETIL = EPAD // P           # 3 slot tiles per expert
NSLOT = E * EPAD           # 3072 total slots
NSLOT_T = NSLOT // P       # 24 slot tiles
f32 = mybir.dt.float32
bf16 = mybir.dt.bfloat16
i32 = mybir.dt.int32
ADD = mybir.AluOpType.add
SUB = mybir.AluOpType.subtract
MUL = mybir.AluOpType.mult


def build_kernel():
    nc = bacc.Bacc(num_devices=N_CORES)

    # ---------------- parameters (host pre-laid-out) ----------------
    xb_s = nc.declare_dram_parameter("xb_s", [TPC, D], bf16, isOutput=False)
    xT_s = nc.declare_dram_parameter("xT_s", [P, DC * TPC], f32, isOutput=False)
    rw_t = nc.declare_dram_parameter("rw_t", [P, DC * E], f32, isOutput=False)
    rb_r = nc.declare_dram_parameter("rb_r", [P, NCOL * E], f32, isOutput=False)
    w1b = nc.declare_dram_parameter("w1b", [E, P, DC * FC * P], bf16, isOutput=False)
    w2b = nc.declare_dram_parameter("w2b", [E, P, FC * DC * P], bf16, isOutput=False)
    b1t = nc.declare_dram_parameter("b1t", [P, E * FC], f32, isOutput=False)
    b2t = nc.declare_dram_parameter("b2t", [P, E * DC], f32, isOutput=False)
    out_shard = nc.declare_dram_parameter("out_shard", [TPC, D], f32, isOutput=True)

    # ---------------- internal DRAM ----------------
    slotmeta_a = nc.dram_tensor("slotmeta_a", [NSLOT, 2], f32)  # k=0 (tok, gate)
    slotmeta_b = nc.dram_tensor("slotmeta_b", [NSLOT, 2], f32)  # k=1 (tok, gate)

    with tile.TileContext(nc) as tc:
        with tc.tile_pool(name="const", bufs=1) as cpool:
            rw_sb = cpool.tile([P, DC, E], f32)
            nc.sync.dma_start(out=rw_sb[:], in_=rw_t.ap().rearrange(
                "p (c e) -> p c e", c=DC))
            rb_sb = cpool.tile([P, NCOL * E], f32)
            nc.sync.dma_start(out=rb_sb[:], in_=rb_r.ap())
            b1_sb = cpool.tile([P, E * FC], f32)
            nc.sync.dma_start(out=b1_sb[:], in_=b1t.ap())
            b2_sb = cpool.tile([P, E * DC], f32)
            nc.sync.dma_start(out=b2_sb[:], in_=b2t.ap())

            ident = cpool.tile([P, P], f32)
            make_identity(nc, ident[:])
            identb = cpool.tile([P, P], bf16)
            nc.vector.tensor_copy(identb[:], ident[:])
            tri = cpool.tile([P, P], f32)
            make_upper_triangular(nc, tri[:], val=1.0, diag=False)  # tri[p,i]=1 iff p<i
            ones_col = cpool.tile([P, 1], f32)
            nc.gpsimd.memset(ones_col[:], 1.0)
            ones_row1 = cpool.tile([1, P], f32)
            nc.gpsimd.memset(ones_row1[:], 1.0)
            toki = cpool.tile([P, NCOL], i32)
            nc.gpsimd.iota(toki[:], pattern=[[P, NCOL]], base=0, channel_multiplier=1)
            tokf = cpool.tile([P, NCOL], f32)
            nc.vector.tensor_copy(tokf[:], toki[:])
            eidxi = cpool.tile([P, E * 2 * NCOL], i32)
            nc.gpsimd.iota(eidxi[:], pattern=[[1, E], [0, 2 * NCOL]], base=0,
                           channel_multiplier=0)
            eidxf = cpool.tile([P, E * 2 * NCOL], f32)
            nc.vector.tensor_copy(eidxf[:], eidxi[:])
            ebase = cpool.tile([P, E * 2 * NCOL], f32)
            nc.vector.tensor_scalar(out=ebase[:], in0=eidxf[:], scalar1=float(EPAD),
                                    scalar2=None, op0=MUL)
            # persistent routing results
            slot_tok = cpool.tile([P, NSLOT_T], i32)
            slot_w = cpool.tile([P, NSLOT_T], f32)

            with tc.tile_pool(name="w1p", bufs=2) as w1p, \
                 tc.tile_pool(name="w2p", bufs=2) as w2p:

                # =========== routing (local tokens only) ===========
                with tc.tile_pool(name="rt", bufs=1) as rt, \
                     tc.tile_pool(name="rtp", bufs=2, space="PSUM") as rtp:
                    # sync queue: xT first (router critical path), split per
                    # dci chunk so the logit matmuls pipeline with the load
                    xT_r = rt.tile([P, DC, TPC], f32)
                    for dci in range(DC):
                        nc.sync.dma_start(
                            out=xT_r[:, dci, :],
                            in_=xT_s.ap().rearrange("p (c t) -> p c t", c=DC)[:, dci, :])
                    # warm up the PE clock (HAM) with junk matmuls on the identity
                    wup = rtp.tile([P, P], f32, space="PSUM", tag="rps")
                    for w in range(24):
                        nc.tensor.matmul(out=wup[:], lhsT=ident[:], rhs=ident[:],
                                         start=(w == 0), stop=(w == 23))
                    # zero the output shard early (scalar queue, idle at start)
                    zot = rt.tile([P, D], f32)
                    nc.vector.memset(zot[:], 0.0)
                    for c in range(NCOL):
                        nc.scalar.dma_start(out=out_shard.ap()[c * P:(c + 1) * P, :],
                                            in_=zot[:])
                    lg_sb = rt.tile([E, TPC], f32)
                    for h in range(2):
                        lgp = rtp.tile([E, TPC // 2], f32, space="PSUM", tag="lgp")
                        for dci in range(DC):
                            nc.tensor.matmul(
                                out=lgp[:], lhsT=rw_sb[:, dci, :],
                                rhs=xT_r[:, dci, h * (TPC // 2):(h + 1) * (TPC // 2)],
                                start=(dci == 0), stop=(dci == DC - 1))
                        nc.vector.tensor_copy(lg_sb[:, h * (TPC // 2):(h + 1) * (TPC // 2)],
                                              lgp[:])
                    # batched top-2: transpose all columns, then wide DVE ops
                    lsb = rt.tile([P, NCOL * E], f32)
                    for c in range(NCOL):
                        ltp = rtp.tile([P, E], f32, space="PSUM", tag="ltp")
                        nc.tensor.transpose(out=ltp[:],
                                            in_=lg_sb[:, c * P:(c + 1) * P],
                                            identity=ident[0:E, 0:E])
                        nc.vector.tensor_copy(lsb[:, c * E:(c + 1) * E], ltp[:])
                    nc.vector.tensor_tensor(out=lsb[:], in0=lsb[:], in1=rb_sb[:],
                                            op=ADD)
                    mxa = rt.tile([P, NCOL * 8], f32)
                    mia = rt.tile([P, NCOL * 8], mybir.dt.uint32)
                    for c in range(NCOL):
                        nc.vector.max_with_indices(mxa[:, c * 8:(c + 1) * 8],
                                                   mia[:, c * 8:(c + 1) * 8],
                                                   lsb[:, c * E:(c + 1) * E])
                    E1f = rt.tile([P, NCOL], f32)
                    E2f = rt.tile([P, NCOL], f32)
                    nc.vector.tensor_copy(E1f[:], mia[:, 0:NCOL * 8:8])
                    nc.vector.tensor_copy(E2f[:], mia[:, 1:NCOL * 8:8])
                    G1 = rt.tile([P, NCOL], f32)
                    G2 = rt.tile([P, NCOL], f32)
                    diff = rt.tile([P, NCOL], f32)
                    nc.vector.tensor_tensor(out=diff[:], in0=mxa[:, 0:NCOL * 8:8],
                                            in1=mxa[:, 1:NCOL * 8:8], op=SUB)
                    nc.scalar.activation(out=G1[:], in_=diff[:],
                                         func=mybir.ActivationFunctionType.Sigmoid)
                    nc.vector.tensor_scalar(out=G2[:], in0=G1[:],
                                            scalar1=-1.0, scalar2=1.0,
                                            op0=MUL, op1=ADD)

                    # ----- batched per-expert ranks -----
                    NW = E * 2 * NCOL      # 128 work columns: e-major, j=(k,c)
                    erep = rt.tile([P, NW], f32)
                    for e in range(E):
                        nc.vector.tensor_copy(erep[:, e * 2 * NCOL:e * 2 * NCOL + NCOL],
                                              E1f[:])
                        nc.vector.tensor_copy(
                            erep[:, e * 2 * NCOL + NCOL:(e + 1) * 2 * NCOL], E2f[:])
                    mask = rt.tile([P, NW], f32)
                    nc.vector.tensor_tensor(out=mask[:], in0=erep[:], in1=eidxf[:],
                                            op=mybir.AluOpType.is_equal)
                    rps = rtp.tile([P, NW], f32, space="PSUM", tag="rps")
                    nc.tensor.matmul(out=rps[:], lhsT=tri[:], rhs=mask[:],
                                     start=True, stop=False)
                    cps = rtp.tile([1, NW], f32, space="PSUM", tag="cps")
                    nc.tensor.matmul(out=cps[:], lhsT=ones_col[:], rhs=mask[:],
                                     start=True, stop=True)
                    ctot = rt.tile([1, NW], f32)
                    nc.vector.tensor_copy(ctot[:], cps[:])
                    cinc = rt.tile([1, NW], f32)
                    for e in range(E):
                        sl = slice(e * 2 * NCOL, (e + 1) * 2 * NCOL)
                        nc.vector.tensor_tensor_scan(
                            out=cinc[:, sl], data0=ctot[:, sl], data1=ctot[:, sl],
                            initial=0.0, op0=ADD, op1=mybir.AluOpType.bypass)
                    cexc = rt.tile([1, NW], f32)
                    nc.vector.tensor_tensor(out=cexc[:], in0=cinc[:], in1=ctot[:],
                                            op=SUB)
                    nc.tensor.matmul(out=rps[:], lhsT=ones_row1[:], rhs=cexc[:],
                                     start=False, stop=True)
                    rank = rt.tile([P, NW], f32)
                    nc.vector.tensor_copy(rank[:], rps[:])
                    kept = rt.tile([P, NW], f32)
                    nc.vector.tensor_scalar(out=kept[:], in0=rank[:],
                                            scalar1=float(EPAD), scalar2=None,
                                            op0=mybir.AluOpType.is_lt)
                    valid = rt.tile([P, NW], f32)
                    nc.vector.tensor_tensor(out=valid[:], in0=mask[:], in1=kept[:],
                                            op=MUL)
                    sel = rt.tile([P, NW], f32)
                    nc.vector.tensor_tensor(out=sel[:], in0=rank[:], in1=ebase[:],
                                            op=ADD)
                    nc.vector.tensor_tensor(out=sel[:], in0=sel[:], in1=valid[:],
                                            op=MUL)
                    # acc[k] = slot index per assignment; vsum = 1 unless overflow
                    acc = rt.tile([P, 2 * NCOL], f32)
                    vsum = rt.tile([P, 2 * NCOL], f32)
                    nc.vector.memset(acc[:], 0.0)
                    nc.vector.memset(vsum[:], 0.0)
                    for e in range(E):
                        sl = slice(e * 2 * NCOL, (e + 1) * 2 * NCOL)
                        nc.vector.tensor_tensor(out=acc[:], in0=acc[:],
                                                in1=sel[:, sl], op=ADD)
                        nc.vector.tensor_tensor(out=vsum[:], in0=vsum[:],
                                                in1=valid[:, sl], op=ADD)
                    # overflow -> push dst far out of bounds (dropped by DMA)
                    nc.vector.tensor_scalar(out=vsum[:], in0=vsum[:],
                                            scalar1=-1.0e6, scalar2=1.0e6,
                                            op0=MUL, op1=ADD)
                    nc.vector.tensor_tensor(out=acc[:], in0=acc[:], in1=vsum[:],
                                            op=ADD)
                    # dst row = (slot%128)*24 + slot//128, per k chain so
                    # k=0 scatters start while k=1 DVE chain still runs
                    # padded slots: token 2048 (out of bounds -> DMA-dropped), gate 0
                    zslot = rt.tile([P, 2 * NSLOT_T], f32)
                    nc.gpsimd.memset(zslot[:, 0:2 * NSLOT_T:2], 2.0 * TPC)
                    nc.gpsimd.memset(zslot[:, 1:2 * NSLOT_T:2], 0.0)
                    for smt in (slotmeta_a, slotmeta_b):
                        nc.gpsimd.dma_start(
                            out=smt.ap().rearrange("(p s) w -> p (s w)", p=P),
                            in_=zslot[:])
                    dsts, pays = [], []
                    for kk in range(2):
                        ak = slice(kk * NCOL, (kk + 1) * NCOL)
                        ri = rt.tile([P, NCOL], i32, tag="ri", bufs=2)
                        nc.vector.tensor_copy(ri[:], acc[:, ak])
                        rand_ = rt.tile([P, NCOL], i32, tag="rand", bufs=2)
                        nc.vector.tensor_scalar(out=rand_[:], in0=ri[:], scalar1=127,
                                                scalar2=None,
                                                op0=mybir.AluOpType.bitwise_and)
                        nc.vector.tensor_scalar(out=rand_[:], in0=rand_[:],
                                                scalar1=NSLOT_T, scalar2=None, op0=MUL)
                        rshr = rt.tile([P, NCOL], i32, tag="rshr", bufs=2)
                        nc.vector.tensor_scalar(out=rshr[:], in0=ri[:], scalar1=7,
                                                scalar2=None,
                                                op0=mybir.AluOpType.logical_shift_right)
                        dst = rt.tile([P, NCOL], i32, tag="dst", bufs=2)
                        nc.vector.tensor_tensor(out=dst[:], in0=rand_[:], in1=rshr[:],
                                                op=ADD)
                        pay = rt.tile([P, 2 * NCOL], f32, tag="pay", bufs=2)
                        nc.vector.tensor_copy(pay[:, 0:2 * NCOL:2], tokf[:])
                        nc.vector.tensor_copy(pay[:, 1:2 * NCOL:2],
                                              G1[:] if kk == 0 else G2[:])
                        dsts.append(dst)
                        pays.append(pay)
                    # alternate the two tables so consecutive queue entries have
                    # no write-after-write dependency and pipeline fully
                    for j in range(NCOL):
                        for kk, smt in ((0, slotmeta_a), (1, slotmeta_b)):
                            nc.gpsimd.indirect_dma_start(
                                out=smt.ap(),
                                out_offset=bass.IndirectOffsetOnAxis(
                                    ap=dsts[kk][:, j:j + 1], axis=0),
                                in_=pays[kk][:, 2 * j:2 * j + 2],
                                in_offset=None,
                                bounds_check=NSLOT - 1,
                                oob_is_err=False,
                            )
                    smfa = rt.tile([P, NSLOT_T, 2], f32)
                    nc.gpsimd.dma_start(out=smfa[:], in_=slotmeta_a.ap().rearrange(
                        "(p s) w -> p s w", p=P))
                    smfb = rt.tile([P, NSLOT_T, 2], f32)
                    nc.gpsimd.dma_start(out=smfb[:], in_=slotmeta_b.ap().rearrange(
                        "(p s) w -> p s w", p=P))
                    # each slot was written by exactly one table; the other
                    # holds the pad (tok=2048, gate=0): tok=min, gate=sum
                    tokm = rt.tile([P, NSLOT_T], f32)
                    nc.vector.tensor_tensor(out=tokm[:], in0=smfa[:, :, 0],
                                            in1=smfb[:, :, 0],
                                            op=mybir.AluOpType.min)
                    nc.vector.tensor_copy(slot_tok[:], tokm[:])
                    nc.vector.tensor_tensor(out=slot_w[:], in0=smfa[:, :, 1],
                                            in1=smfb[:, :, 1], op=ADD)

                # =========== expert FFN + scatter-add combine ===========
                with tc.tile_pool(name="ffn", bufs=1) as ffn, \
                     tc.tile_pool(name="xgp", bufs=2) as xgp, \
                     tc.tile_pool(name="xtp", bufs=2) as xtp, \
                     tc.tile_pool(name="ffp", bufs=2, space="PSUM") as ffp:

                    def issue_gather(e):
                        xg3 = xgp.tile([P, ETIL, D], bf16, tag="xg3")
                        for t in range(ETIL):
                            nc.gpsimd.indirect_dma_start(
                                out=xg3[:, t, :], out_offset=None,
                                in_=xb_s.ap(),
                                in_offset=bass.IndirectOffsetOnAxis(
                                    ap=slot_tok[:, e * ETIL + t:e * ETIL + t + 1],
                                    axis=0),
                                bounds_check=TPC - 1,
                                oob_is_err=False,
                            )
                        return xg3

                    def xgT_build(xg3):
                        # 8 bf16 transposes into one 1-bank psum tile, then a
                        # single wide DVE drain per slot tile
                        xgT = xtp.tile([P, DC, EPAD], bf16, tag="xgT")
                        for t in range(ETIL):
                            tpb = ffp.tile([P, DC, P], bf16, space="PSUM", tag="tpb")
                            for dci in range(DC):
                                nc.tensor.transpose(
                                    out=tpb[:, dci, :],
                                    in_=xg3[:, t, dci * P:(dci + 1) * P],
                                    identity=identb[:])
                            nc.vector.tensor_copy(xgT[:, :, t * P:(t + 1) * P],
                                                  tpb[:])
                        return xgT

                    xg3_cur = issue_gather(0)
                    xgT_cur = xgT_build(xg3_cur)
                    for e in range(E):
                        w1t = w1p.tile([P, DC, FC, P], bf16, tag="w1")
                        nc.sync.dma_start(out=w1t[:], in_=w1b.ap()[e].rearrange(
                            "p (a b q) -> p a b q", a=DC, b=FC))
                        w2t = w2p.tile([P, FC, DC, P], bf16, tag="w2")
                        nc.sync.dma_start(out=w2t[:], in_=w2b.ap()[e].rearrange(
                            "p (a b q) -> p a b q", a=FC, b=DC))
                        if e + 1 < E:
                            xg3_nxt = issue_gather(e + 1)

                        # mm1 + gelu -> hT
                        hT = ffn.tile([P, FC, EPAD], bf16, tag="hT", bufs=1)
                        for fci in range(FC):
                            hp = ffp.tile([P, EPAD], f32, space="PSUM", tag="hp")
                            for dci in range(DC):
                                nc.tensor.matmul(out=hp[:],
                                                 lhsT=w1t[:, dci, fci, :],
                                                 rhs=xgT_cur[:, dci, :],
                                                 start=(dci == 0), stop=(dci == DC - 1))
                            nc.scalar.activation(out=hT[:, fci, :], in_=hp[:],
                                                 func=mybir.ActivationFunctionType.Gelu,
                                                 bias=b1_sb[:, e * FC + fci:e * FC + fci + 1],
                                                 scale=1.0)
                        # next expert's input transposes; their DVE drains
                        # overlap this expert's mm2
                        if e + 1 < E:
                            xgT_nxt = xgT_build(xg3_nxt)

                        # mm2 (+bias) -> oT
                        oT = ffn.tile([P, DC, EPAD], bf16, tag="oT", bufs=1)
                        for dci in range(DC):
                            op = ffp.tile([P, EPAD], f32, space="PSUM", tag="op")
                            for fci in range(FC):
                                nc.tensor.matmul(out=op[:],
                                                 lhsT=w2t[:, fci, dci, :],
                                                 rhs=hT[:, fci, :],
                                                 start=(fci == 0), stop=(fci == FC - 1))
                            nc.vector.tensor_scalar(
                                out=oT[:, dci, :], in0=op[:],
                                scalar1=b2_sb[:, e * DC + dci:e * DC + dci + 1],
                                scalar2=None, op0=ADD)
                        # transpose back + gate (one DVE drain per slot tile),
                        # scatter-add into the output shard
                        ow3 = ffn.tile([P, ETIL, D], f32, tag="ow3", bufs=2)
                        for cc in range(ETIL):
                            tp2b = ffp.tile([P, DC, P], bf16, space="PSUM", tag="tp2b")
                            for dci in range(DC):
                                nc.tensor.transpose(
                                    out=tp2b[:, dci, :],
                                    in_=oT[:, dci, cc * P:(cc + 1) * P],
                                    identity=identb[:])
                            nc.vector.tensor_scalar(
                                out=ow3[:, cc, :], in0=tp2b[:],
                                scalar1=slot_w[:, e * ETIL + cc:e * ETIL + cc + 1],
                                scalar2=None, op0=MUL)
                            nc.gpsimd.indirect_dma_start(
                                out=out_shard.ap(),
                                out_offset=bass.IndirectOffsetOnAxis(
                                    ap=slot_tok[:, e * ETIL + cc:e * ETIL + cc + 1],
                                    axis=0),
                                in_=ow3[:, cc, :],
                                in_offset=None,
                                bounds_check=TPC - 1,
                                oob_is_err=False,
                                compute_op=ADD,
                            )
                        if e + 1 < E:
                            xg3_cur, xgT_cur = xg3_nxt, xgT_nxt

    nc.finalize()
    return nc


_NC_CACHE = None
TRACE = False
LAST_EXEC_NS = None
LAST_TRACE_DIR = None


def kernel(x, router_w, router_b, w1, b1, w2, b2):
    global _NC_CACHE, LAST_EXEC_NS, LAST_TRACE_DIR
    import ml_dtypes
    bf = ml_dtypes.bfloat16

    x = np.ascontiguousarray(np.asarray(x, np.float32))
    router_w = np.ascontiguousarray(np.asarray(router_w, np.float32))
    router_b = np.asarray(router_b, np.float32)
    w1 = np.asarray(w1, np.float32)
    b1 = np.asarray(b1, np.float32)
    w2 = np.asarray(w2, np.float32)
    b2 = np.asarray(b2, np.float32)

    xf = x.reshape(T, D)
    rb_r = np.ascontiguousarray(np.tile(router_b, (P, NCOL)))
    rw_t = np.ascontiguousarray(
        router_w.reshape(DC, P, E).transpose(1, 0, 2).reshape(P, DC * E))
    w1b = np.ascontiguousarray(
        w1.reshape(E, DC, P, FC, P).transpose(0, 2, 1, 3, 4)
        .reshape(E, P, DC * FC * P).astype(bf))
    w2b = np.ascontiguousarray(
        w2.reshape(E, FC, P, DC, P).transpose(0, 2, 1, 3, 4)
        .reshape(E, P, FC * DC * P).astype(bf))
    b1t = np.ascontiguousarray(
        b1.reshape(E, FC, P).transpose(2, 0, 1).reshape(P, E * FC))
    b2t = np.ascontiguousarray(
        b2.reshape(E, DC, P).transpose(2, 0, 1).reshape(P, E * DC))

    in_maps = []
    for c in range(N_CORES):
        xs = np.ascontiguousarray(xf[c * TPC:(c + 1) * TPC])
        xT = np.ascontiguousarray(
            xs.T.reshape(DC, P, TPC).transpose(1, 0, 2).reshape(P, DC * TPC))
        in_maps.append({
            "xb_s": xs.astype(bf),
            "xT_s": xT,
            "rw_t": rw_t,
            "rb_r": rb_r,
            "w1b": w1b,
            "w2b": w2b,
            "b1t": b1t,
            "b2t": b2t,
        })

    if _NC_CACHE is None:
        _NC_CACHE = build_kernel()
    import tempfile
    td = tempfile.mkdtemp(prefix="moe_trace_") if TRACE else None
    res = run_bass_kernel_spmd(_NC_CACHE, in_maps, list(range(N_CORES)),
                               trace=TRACE, tmpdir=td)
    LAST_EXEC_NS = getattr(res, "exec_time_ns", None)
    LAST_TRACE_DIR = td
    out = np.concatenate([res.results[c]["out_shard"] for c in range(N_CORES)], axis=0)
    return out.reshape(B, S, D)
